# revision 2
# baseline (speedup 1.0000x reference)
"""Causal self-attention (B=2, T=2048, E=1024, 16 heads) on 8 TRN2 NeuronCores.

Sharding (Megatron-style, zero device-side collectives):
  core c in 0..7 -> batch b = c//4, head group hg = c%4 (4 heads, 256 head-dims).
  Each core computes, for its batch and its 4 heads:
    qT/kT = (w_q|w_k)^T x^T   (transposed layout: [head_dim, T])
    v     = x w_v             (natural layout: [T, head_dim], + ones column)
    sT    = kT^T-block matmuls -> [tk, tq] score blocks (causal blocks only)
    expS  = exp(sT/8) * mask  (no max-subtraction: scores are O(1) by construction)
    yT    = v_plus^T @ expS   -> [65, tq]; row 64 accumulates the softmax row-sums
    y_norm= yT[0:64] * broadcast(1/rowsum)   (broadcast via K=1 matmul on PE)
    out_c = y_norm^T w_proj[rows of its heads]  -> partial [T, E]
  Host: out[b] = sum of the 4 partials + b_proj + b_v @ w_proj.
  b_k is dropped (softmax is invariant to per-row constants); b_q is applied
  on-device via the activation bias port; b_v is folded into the output bias.

All matmuls run in float32r (TF32-like, 1 cycle/row at N>=256; ~1.5e-4 rel err).
"""

import numpy as np

N_HEAD = 16
E = 1024
B, T = 2, 2048
HD = E // N_HEAD          # 64
N_CORES = 8
HPC = 4                   # heads per core
DJ = HPC * HD             # 256 head-dim columns per core
ET = E // 128             # 8  e-tiles
TT = T // 128             # 16 t-tiles
TC = T // 512             # 4  t-chunks
SCALE = 1.0 / np.sqrt(HD)  # 0.125

_STATE = {}


def _build_nc():
    import concourse.tile as tile
    from concourse import mybir
    from concourse.bacc import Bacc

    f32 = mybir.dt.float32
    f32r = mybir.dt.float32r
    AF = mybir.ActivationFunctionType

    nc = Bacc()
    xT_d = nc.dram_tensor("xT", [E, T], f32r, kind="ExternalInput")
    wqk_d = nc.dram_tensor("wqk", [E, 2 * DJ], f32r, kind="ExternalInput")
    wv_d = nc.dram_tensor("wv", [E, DJ], f32r, kind="ExternalInput")
    wp_d = nc.dram_tensor("wp", [DJ, E], f32r, kind="ExternalInput")
    bq_d = nc.dram_tensor("bq", [128, 2], f32, kind="ExternalInput")
    mask_d = nc.dram_tensor("mask", [4, 128, 512], f32, kind="ExternalInput")
    ones4_d = nc.dram_tensor("ones4", [128, HPC, 1], f32r, kind="ExternalInput")
    ones1_d = nc.dram_tensor("ones1", [1, 128], f32r, kind="ExternalInput")
    out_d = nc.dram_tensor("out", [T, E], f32, kind="ExternalOutput")

    with tile.TileContext(nc) as tc:
        with (
            tc.tile_pool(name="xw", bufs=1) as xw,          # persistent inputs
            tc.tile_pool(name="qkv", bufs=1) as qkv,        # persistent qT/kT/v/yT
            tc.tile_pool(name="es", bufs=6) as esp,         # exp(score) blocks
            tc.tile_pool(name="nrm", bufs=2) as nrm,        # norm scratch
            tc.tile_pool(name="ob", bufs=3) as obp,         # output staging
            tc.tile_pool(name="ps", bufs=3, space="PSUM") as ps,
            tc.tile_pool(name="psy", bufs=2, space="PSUM") as psy,
        ):
            # ---- load persistent inputs ----
            xT_sb = []
            wqk_sb = []
            wv_sb = []
            for et in range(ET):
                t = xw.tile([128, T], f32r, tag=f"xT{et}")
                nc.sync.dma_start(t[:], xT_d[128 * et : 128 * (et + 1), :])
                xT_sb.append(t)
                t = xw.tile([128, 2 * DJ], f32r, tag=f"wqk{et}")
                nc.sync.dma_start(t[:], wqk_d[128 * et : 128 * (et + 1), :])
                wqk_sb.append(t)
                t = xw.tile([128, DJ], f32r, tag=f"wv{et}")
                nc.sync.dma_start(t[:], wv_d[128 * et : 128 * (et + 1), :])
                wv_sb.append(t)
            wp_sb = []
            for kt in range(2):
                t = xw.tile([128, E], f32r, tag=f"wp{kt}")
                nc.sync.dma_start(t[:], wp_d[128 * kt : 128 * (kt + 1), :])
                wp_sb.append(t)
            bq_sb = xw.tile([128, 2], f32, tag="bq")
            nc.sync.dma_start(bq_sb[:], bq_d[:])
            mask_sb = []
            for m in range(4):
                t = xw.tile([128, 512], f32, tag=f"mask{m}")
                nc.sync.dma_start(t[:], mask_d[m])
                mask_sb.append(t)
            ones4_sb = xw.tile([128, HPC, 1], f32r, tag="ones4")
            nc.sync.dma_start(ones4_sb[:], ones4_d[:])
            ones1_sb = xw.tile([1, 128], f32r, tag="ones1")
            nc.sync.dma_start(ones1_sb[:], ones1_d[:])

            # persistent intermediates
            qT_sb = [qkv.tile([128, T], f32r, tag=f"qT{i}", name=f"qT{i}") for i in range(2)]
            kT_sb = [qkv.tile([128, T], f32r, tag=f"kT{i}", name=f"kT{i}") for i in range(2)]
            v_sb = [qkv.tile([128, HPC, HD + 1], f32r, tag=f"v{i}", name=f"v{i}") for i in range(TT)]
            yT_sb = [qkv.tile([128, T], f32r, tag=f"yT{i}", name=f"yT{i}") for i in range(2)]

            # ---- phase 1: qT / kT  (transposed projections) ----
            # psum [j=128, t=512] = sum_e wqk[e, j-tile]^T @ xT[e, t-chunk]
            for jt in range(4):          # 0,1 -> q ; 2,3 -> k
                for ci in range(TC):
                    acc = ps.tile([128, 512], f32, tag="mm")
                    for et in range(ET):
                        nc.tensor.matmul(
                            acc[:],
                            wqk_sb[et][:, 128 * jt : 128 * (jt + 1)],
                            xT_sb[et][:, 512 * ci : 512 * (ci + 1)],
                            start=(et == 0),
                            stop=(et == ET - 1),
                        )
                    if jt < 2:
                        # q: add bias while copying out of PSUM (ACT engine)
                        nc.scalar.activation(
                            out=qT_sb[jt][:, 512 * ci : 512 * (ci + 1)],
                            in_=acc[:],
                            func=AF.Identity,
                            bias=bq_sb[:, jt : jt + 1],
                        )
                    else:
                        nc.vector.tensor_copy(
                            kT_sb[jt - 2][:, 512 * ci : 512 * (ci + 1)], acc[:]
                        )

            # ---- phase 2: v (natural layout) + ones column ----
            for tt in range(TT):
                acc = ps.tile([128, DJ], f32, tag="mm")
                for et in range(ET):
                    nc.tensor.matmul(
                        acc[:],
                        xT_sb[et][:, 128 * tt : 128 * (tt + 1)],
                        wv_sb[et][:],
                        start=(et == 0),
                        stop=(et == ET - 1),
                    )
                nc.vector.tensor_copy(
                    v_sb[tt][:, :, 0:HD],
                    acc[:].rearrange("p (h d) -> p h d", h=HPC),
                )
                nc.vector.tensor_copy(v_sb[tt][:, :, HD : HD + 1], ones4_sb[:])

            # ---- phase 3: attention per (head, tq-chunk) ----
            for h in range(HPC):
                kth = kT_sb[h // 2]
                qth = qT_sb[h // 2]
                r0 = HD * (h % 2)
                for ci in range(TC):
                    nj = 4 * ci + 4
                    yacc = psy.tile([HD + 1, 512], f32, tag="y")
                    for j in range(nj):
                        sacc = ps.tile([128, 512], f32, tag="s")
                        nc.tensor.matmul(
                            sacc[:],
                            kth[r0 : r0 + HD, 128 * j : 128 * (j + 1)],
                            qth[r0 : r0 + HD, 512 * ci : 512 * (ci + 1)],
                        )
                        es = esp.tile([128, 512], f32r, tag="es")
                        nc.scalar.activation(
                            out=es[:], in_=sacc[:], func=AF.Exp, scale=float(SCALE)
                        )
                        if j >= 4 * ci:
                            nc.vector.tensor_mul(es[:], es[:], mask_sb[j - 4 * ci][:])
                        nc.tensor.matmul(
                            yacc[:],
                            v_sb[j][:, h, :],
                            es[:],
                            start=(j == 0),
                            stop=(j == nj - 1),
                        )
                    # normalize: yT[0:64] * (1/rowsum) ; broadcast via K=1 matmul
                    rrow = nrm.tile([1, 512], f32r, tag="rr")
                    with nc.allow_low_precision(reason="softmax reciprocal"):
                        nc.vector.reciprocal(rrow[:], yacc[HD : HD + 1, :])
                    bacc = ps.tile([HD, 512], f32, tag="s")
                    nc.tensor.matmul(bacc[:], ones1_sb[:, 0:HD], rrow[:])
                    bs = nrm.tile([HD, 512], f32, tag="bs")
                    nc.vector.tensor_copy(bs[:], bacc[:])
                    nc.vector.tensor_mul(
                        yT_sb[h // 2][r0 : r0 + HD, 512 * ci : 512 * (ci + 1)],
                        yacc[0:HD, :],
                        bs[:],
                    )

            # ---- phase 4: projection ----
            for tt in range(TT):
                ob = obp.tile([128, E], f32, tag="ob")
                for nk in range(2):
                    acc = ps.tile([128, 512], f32, tag="mm")
                    for kt in range(2):
                        nc.tensor.matmul(
                            acc[:],
                            yT_sb[kt][:, 128 * tt : 128 * (tt + 1)],
                            wp_sb[kt][:, 512 * nk : 512 * (nk + 1)],
                            start=(kt == 0),
                            stop=(kt == 1),
                        )
                    nc.vector.tensor_copy(ob[:, 512 * nk : 512 * (nk + 1)], acc[:])
                nc.sync.dma_start(out_d[128 * tt : 128 * (tt + 1), :], ob[:])

    nc.finalize()
    return nc


def _host_constants():
    # diagonal causal masks: mask[m][r, c] = 1.0 if c >= r + 128*m else 0
    masks = np.zeros((4, 128, 512), dtype=np.float32)
    r = np.arange(128)[:, None]
    c = np.arange(512)[None, :]
    for m in range(4):
        masks[m] = (c >= r + 128 * m).astype(np.float32)
    ones4 = np.ones((128, HPC, 1), dtype=np.float32)
    ones1 = np.ones((1, 128), dtype=np.float32)
    return masks, ones4, ones1


def _make_in_maps(x, w_qkv, b_qkv):
    masks, ones4, ones1 = _host_constants()
    in_maps = []
    for c in range(N_CORES):
        b, hg = divmod(c, HPC)
        j0 = DJ * hg
        xT = np.ascontiguousarray(np.asarray(x[b], dtype=np.float32).T)
        wq = w_qkv[:, j0 : j0 + DJ]
        wk = w_qkv[:, E + j0 : E + j0 + DJ]
        wqk = np.ascontiguousarray(
            np.concatenate([wq, wk], axis=1), dtype=np.float32
        )
        wv = np.ascontiguousarray(w_qkv[:, 2 * E + j0 : 2 * E + j0 + DJ],
                                  dtype=np.float32)
        bq = np.ascontiguousarray(
            np.asarray(b_qkv[j0 : j0 + DJ], dtype=np.float32).reshape(2, 128).T
        )
        in_maps.append(
            {
                "xT": xT,
                "wqk": wqk,
                "wv": wv,
                "wp": None,  # filled below (needs w_proj)
                "bq": bq,
                "mask": masks,
                "ones4": ones4,
                "ones1": ones1,
            }
        )
    return in_maps


def kernel(x, w_qkv, b_qkv, w_proj, b_proj):
    from concourse.bass_utils import run_bass_kernel_spmd

    x = np.asarray(x, dtype=np.float32)
    w_qkv = np.asarray(w_qkv, dtype=np.float32)
    b_qkv = np.asarray(b_qkv, dtype=np.float32)
    w_proj = np.asarray(w_proj, dtype=np.float32)
    b_proj = np.asarray(b_proj, dtype=np.float32)

    if "nc" not in _STATE:
        _STATE["nc"] = _build_nc()
    nc = _STATE["nc"]

    in_maps = _make_in_maps(x, w_qkv, b_qkv)
    for c in range(N_CORES):
        _, hg = divmod(c, HPC)
        j0 = DJ * hg
        in_maps[c]["wp"] = np.ascontiguousarray(w_proj[j0 : j0 + DJ, :],
                                                dtype=np.float32)

    res = run_bass_kernel_spmd(nc, in_maps, core_ids=list(range(N_CORES)))

    out = np.zeros((B, T, E), dtype=np.float32)
    for c in range(N_CORES):
        out[c // HPC] += res.results[c]["out"]
    # fold b_v through the projection; b_k cancels inside softmax
    bias = b_proj + b_qkv[2 * E :] @ w_proj
    out += bias[None, None, :]
    return out


# revision 4
# speedup vs baseline: 49.5573x; 49.5573x over previous
"""Causal self-attention (B=2, T=2048, E=1024, 16 heads) on 8 TRN2 NeuronCores.

Sharding (Megatron-style, zero device-side collectives):
  core c in 0..7 -> batch b = c//4, head group hg = c%4 (4 heads, 256 head-dims).
  Each core computes, for its batch and its 4 heads:
    qT/kT = (w_q|w_k)^T x^T   (transposed layout: [head_dim, T])
    v     = x w_v             (natural layout: [T, head_dim], + ones column)
    sT    = kT^T-block matmuls -> [tk, tq] score blocks (causal blocks only)
    expS  = exp(sT/8) * mask  (no max-subtraction: scores are O(1) by construction)
    yT    = v_plus^T @ expS   -> [65, tq]; row 64 accumulates the softmax row-sums
    y_norm= yT[0:64] * broadcast(1/rowsum)   (broadcast via K=1 matmul on PE)
    out_c = y_norm^T w_proj[rows of its heads]  -> partial [T, E]
  Host: out[b] = sum of the 4 partials + b_proj + b_v @ w_proj.
  b_k is dropped (softmax is invariant to per-row constants); b_q is applied
  on-device via the activation bias port; b_v is folded into the output bias.

All matmuls run in float32r (TF32-like, 1 cycle/row at N>=256; ~1.5e-4 rel err).
"""

import numpy as np

N_HEAD = 16
E = 1024
B, T = 2, 2048
HD = E // N_HEAD          # 64
N_CORES = 8
HPC = 4                   # heads per core
DJ = HPC * HD             # 256 head-dim columns per core
ET = E // 128             # 8  e-tiles
TT = T // 128             # 16 t-tiles
TC = T // 512             # 4  t-chunks
SCALE = 1.0 / np.sqrt(HD)  # 0.125

_STATE = {}


def _build_nc():
    import concourse.tile as tile
    from concourse import mybir
    from concourse.bacc import Bacc

    f32 = mybir.dt.float32
    f32r = mybir.dt.float32r
    AF = mybir.ActivationFunctionType

    nc = Bacc()
    xT_d = nc.dram_tensor("xT", [E, T], f32r, kind="ExternalInput")
    wqk_d = nc.dram_tensor("wqk", [E, 2 * DJ], f32r, kind="ExternalInput")
    wv_d = nc.dram_tensor("wv", [E, DJ], f32r, kind="ExternalInput")
    wp_d = nc.dram_tensor("wp", [DJ, E], f32r, kind="ExternalInput")
    bq_d = nc.dram_tensor("bq", [128, 2], f32, kind="ExternalInput")
    mask_d = nc.dram_tensor("mask", [4, 128, 512], f32, kind="ExternalInput")
    ones4_d = nc.dram_tensor("ones4", [128, HPC, 1], f32r, kind="ExternalInput")
    ones1_d = nc.dram_tensor("ones1", [1, 128], f32r, kind="ExternalInput")
    out_d = nc.dram_tensor("out", [T, E], f32, kind="ExternalOutput")

    with tile.TileContext(nc) as tc:
        with (
            tc.tile_pool(name="xw", bufs=1) as xw,          # persistent inputs
            tc.tile_pool(name="qkv", bufs=1) as qkv,        # persistent qT/kT/v/yT
            tc.tile_pool(name="es", bufs=6) as esp,         # exp(score) blocks
            tc.tile_pool(name="nrm", bufs=2) as nrm,        # norm scratch
            tc.tile_pool(name="ob", bufs=3) as obp,         # output staging
            tc.tile_pool(name="ps", bufs=3, space="PSUM") as ps,
            tc.tile_pool(name="psy", bufs=2, space="PSUM") as psy,
        ):
            # ---- load persistent inputs ----
            xT_sb = []
            wqk_sb = []
            wv_sb = []
            for et in range(ET):
                t = xw.tile([128, T], f32r, tag=f"xT{et}")
                nc.sync.dma_start(t[:], xT_d[128 * et : 128 * (et + 1), :])
                xT_sb.append(t)
                t = xw.tile([128, 2 * DJ], f32r, tag=f"wqk{et}")
                nc.sync.dma_start(t[:], wqk_d[128 * et : 128 * (et + 1), :])
                wqk_sb.append(t)
                t = xw.tile([128, DJ], f32r, tag=f"wv{et}")
                nc.sync.dma_start(t[:], wv_d[128 * et : 128 * (et + 1), :])
                wv_sb.append(t)
            wp_sb = []
            for kt in range(2):
                t = xw.tile([128, E], f32r, tag=f"wp{kt}")
                nc.sync.dma_start(t[:], wp_d[128 * kt : 128 * (kt + 1), :])
                wp_sb.append(t)
            bq_sb = xw.tile([128, 2], f32, tag="bq")
            nc.sync.dma_start(bq_sb[:], bq_d[:])
            mask_sb = []
            for m in range(4):
                t = xw.tile([128, 512], f32, tag=f"mask{m}")
                nc.sync.dma_start(t[:], mask_d[m])
                mask_sb.append(t)
            ones4_sb = xw.tile([128, HPC, 1], f32r, tag="ones4")
            nc.sync.dma_start(ones4_sb[:], ones4_d[:])
            ones1_sb = xw.tile([1, 128], f32r, tag="ones1")
            nc.sync.dma_start(ones1_sb[:], ones1_d[:])

            # persistent intermediates
            qT_sb = [qkv.tile([128, T], f32r, tag=f"qT{i}", name=f"qT{i}") for i in range(2)]
            kT_sb = [qkv.tile([128, T], f32r, tag=f"kT{i}", name=f"kT{i}") for i in range(2)]
            v_sb = [qkv.tile([128, HPC, HD + 1], f32r, tag=f"v{i}", name=f"v{i}") for i in range(TT)]
            yT_sb = [qkv.tile([128, T], f32r, tag=f"yT{i}", name=f"yT{i}") for i in range(2)]

            # ---- phase 1: qT / kT  (transposed projections) ----
            # psum [j=128, t=512] = sum_e wqk[e, j-tile]^T @ xT[e, t-chunk]
            for jt in range(4):          # 0,1 -> q ; 2,3 -> k
                for ci in range(TC):
                    acc = ps.tile([128, 512], f32, tag="mm")
                    for et in range(ET):
                        nc.tensor.matmul(
                            acc[:],
                            wqk_sb[et][:, 128 * jt : 128 * (jt + 1)],
                            xT_sb[et][:, 512 * ci : 512 * (ci + 1)],
                            start=(et == 0),
                            stop=(et == ET - 1),
                        )
                    if jt < 2:
                        # q: add bias while copying out of PSUM (ACT engine)
                        nc.scalar.activation(
                            out=qT_sb[jt][:, 512 * ci : 512 * (ci + 1)],
                            in_=acc[:],
                            func=AF.Identity,
                            bias=bq_sb[:, jt : jt + 1],
                        )
                    else:
                        nc.vector.tensor_copy(
                            kT_sb[jt - 2][:, 512 * ci : 512 * (ci + 1)], acc[:]
                        )

            # ---- phase 2: v (natural layout) + ones column ----
            for tt in range(TT):
                acc = ps.tile([128, DJ], f32, tag="mm")
                for et in range(ET):
                    nc.tensor.matmul(
                        acc[:],
                        xT_sb[et][:, 128 * tt : 128 * (tt + 1)],
                        wv_sb[et][:],
                        start=(et == 0),
                        stop=(et == ET - 1),
                    )
                nc.vector.tensor_copy(
                    v_sb[tt][:, :, 0:HD],
                    acc[:].rearrange("p (h d) -> p h d", h=HPC),
                )
                nc.vector.tensor_copy(v_sb[tt][:, :, HD : HD + 1], ones4_sb[:])

            # ---- phase 3: attention per (head, tq-chunk) ----
            for h in range(HPC):
                kth = kT_sb[h // 2]
                qth = qT_sb[h // 2]
                r0 = HD * (h % 2)
                for ci in range(TC):
                    nj = 4 * ci + 4
                    yacc = psy.tile([HD + 1, 512], f32, tag="y")
                    for j in range(nj):
                        sacc = ps.tile([128, 512], f32, tag="s")
                        nc.tensor.matmul(
                            sacc[:],
                            kth[r0 : r0 + HD, 128 * j : 128 * (j + 1)],
                            qth[r0 : r0 + HD, 512 * ci : 512 * (ci + 1)],
                        )
                        es = esp.tile([128, 512], f32r, tag="es")
                        nc.scalar.activation(
                            out=es[:], in_=sacc[:], func=AF.Exp, scale=float(SCALE)
                        )
                        if j >= 4 * ci:
                            nc.vector.tensor_mul(es[:], es[:], mask_sb[j - 4 * ci][:])
                        nc.tensor.matmul(
                            yacc[:],
                            v_sb[j][:, h, :],
                            es[:],
                            start=(j == 0),
                            stop=(j == nj - 1),
                        )
                    # normalize: yT[0:64] * (1/rowsum) ; broadcast via K=1 matmul
                    rrow = nrm.tile([1, 512], f32r, tag="rr")
                    with nc.allow_low_precision(reason="softmax reciprocal"):
                        nc.vector.reciprocal(rrow[:], yacc[HD : HD + 1, :])
                    bacc = ps.tile([HD, 512], f32, tag="s")
                    nc.tensor.matmul(bacc[:], ones1_sb[:, 0:HD], rrow[:])
                    bs = nrm.tile([HD, 512], f32, tag="bs")
                    nc.vector.tensor_copy(bs[:], bacc[:])
                    nc.vector.tensor_mul(
                        yT_sb[h // 2][r0 : r0 + HD, 512 * ci : 512 * (ci + 1)],
                        yacc[0:HD, :],
                        bs[:],
                    )

            # ---- phase 4: projection ----
            for tt in range(TT):
                ob = obp.tile([128, E], f32, tag="ob")
                for nk in range(2):
                    acc = ps.tile([128, 512], f32, tag="mm")
                    for kt in range(2):
                        nc.tensor.matmul(
                            acc[:],
                            yT_sb[kt][:, 128 * tt : 128 * (tt + 1)],
                            wp_sb[kt][:, 512 * nk : 512 * (nk + 1)],
                            start=(kt == 0),
                            stop=(kt == 1),
                        )
                    nc.vector.tensor_copy(ob[:, 512 * nk : 512 * (nk + 1)], acc[:])
                nc.sync.dma_start(out_d[128 * tt : 128 * (tt + 1), :], ob[:])

    nc.finalize()
    return nc


def _host_constants():
    # diagonal causal masks: mask[m][r, c] = 1.0 if c >= r + 128*m else 0
    masks = np.zeros((4, 128, 512), dtype=np.float32)
    r = np.arange(128)[:, None]
    c = np.arange(512)[None, :]
    for m in range(4):
        masks[m] = (c >= r + 128 * m).astype(np.float32)
    ones4 = np.ones((128, HPC, 1), dtype=np.float32)
    ones1 = np.ones((1, 128), dtype=np.float32)
    return masks, ones4, ones1


def _make_in_maps(x, w_qkv, b_qkv):
    masks, ones4, ones1 = _host_constants()
    in_maps = []
    for c in range(N_CORES):
        b, hg = divmod(c, HPC)
        j0 = DJ * hg
        xT = np.ascontiguousarray(np.asarray(x[b], dtype=np.float32).T)
        wq = w_qkv[:, j0 : j0 + DJ]
        wk = w_qkv[:, E + j0 : E + j0 + DJ]
        wqk = np.ascontiguousarray(
            np.concatenate([wq, wk], axis=1), dtype=np.float32
        )
        wv = np.ascontiguousarray(w_qkv[:, 2 * E + j0 : 2 * E + j0 + DJ],
                                  dtype=np.float32)
        bq = np.ascontiguousarray(
            np.asarray(b_qkv[j0 : j0 + DJ], dtype=np.float32).reshape(2, 128).T
        )
        in_maps.append(
            {
                "xT": xT,
                "wqk": wqk,
                "wv": wv,
                "wp": None,  # filled below (needs w_proj)
                "bq": bq,
                "mask": masks,
                "ones4": ones4,
                "ones1": ones1,
            }
        )
    return in_maps


def _get_exec():
    """Build the Bass module and a cached jitted SPMD callable (once)."""
    if "exec" in _STATE:
        return _STATE["exec"]

    import jax
    from concourse import bass2jax, mybir
    from jax.experimental.shard_map import shard_map
    from jax.sharding import Mesh, PartitionSpec

    nc = _build_nc()
    bass2jax.install_neuronx_cc_hook()

    partition_name = (
        nc.partition_id_tensor.name if nc.partition_id_tensor else None
    )
    in_names = []
    out_names = []
    out_avals = []
    zero_outs = []
    for alloc in nc.m.functions[0].allocations:
        if not isinstance(alloc, mybir.MemoryLocationSet):
            continue
        name = alloc.memorylocations[0].name
        if alloc.kind == "ExternalInput":
            if name != partition_name:
                in_names.append(name)
        elif alloc.kind == "ExternalOutput":
            shape = tuple(alloc.tensor_shape)
            dtype = mybir.dt.np(alloc.dtype)
            out_names.append(name)
            out_avals.append(jax.core.ShapedArray(shape, dtype))
            zero_outs.append(np.zeros(shape, dtype))
    n_params = len(in_names)
    all_names = in_names + out_names
    if partition_name is not None:
        all_names = all_names + [partition_name]

    def _body(*args):
        operands = list(args)
        if partition_name is not None:
            operands.append(bass2jax.partition_id_tensor())
        outs = bass2jax._bass_exec_p.bind(
            *operands,
            out_avals=tuple(out_avals),
            in_names=tuple(all_names),
            out_names=tuple(out_names),
            lowering_input_output_aliases=(),
            sim_require_finite=True,
            sim_require_nnan=True,
            nc=nc,
        )
        return tuple(outs)

    devices = jax.devices()[:N_CORES]
    mesh = Mesh(np.asarray(devices), ("core",))
    n_all = n_params + len(out_names)
    sharded = jax.jit(
        shard_map(
            _body,
            mesh=mesh,
            in_specs=(PartitionSpec("core"),) * n_all,
            out_specs=(PartitionSpec("core"),) * len(out_names),
            check_rep=False,
        ),
        keep_unused=True,
    )

    state = {
        "jax": jax,
        "sharded": sharded,
        "in_names": in_names,
        "out_names": out_names,
        "out_avals": out_avals,
        "zeros_dev": [
            jax.device_put(
                np.zeros((N_CORES * z.shape[0], *z.shape[1:]), z.dtype)
            )
            for z in zero_outs
        ],
    }
    _STATE["exec"] = state
    return state


def _concat_inputs(in_maps):
    st = _get_exec()
    return [
        np.concatenate([np.asarray(in_maps[c][name]) for c in range(N_CORES)], axis=0)
        for name in st["in_names"]
    ]


def _run_device(concat_in):
    """concat_in: list of global (8*dim0, ...) arrays (np or jax). Returns
    list of per-core output dicts."""
    st = _get_exec()
    out_arrs = st["sharded"](*concat_in, *st["zeros_dev"])
    res = []
    for c in range(N_CORES):
        d = {}
        for i, name in enumerate(st["out_names"]):
            shp = st["out_avals"][i].shape
            d[name] = np.asarray(out_arrs[i]).reshape(N_CORES, *shp)[c]
        res.append(d)
    return res


def kernel(x, w_qkv, b_qkv, w_proj, b_proj):
    x = np.asarray(x, dtype=np.float32)
    w_qkv = np.asarray(w_qkv, dtype=np.float32)
    b_qkv = np.asarray(b_qkv, dtype=np.float32)
    w_proj = np.asarray(w_proj, dtype=np.float32)
    b_proj = np.asarray(b_proj, dtype=np.float32)

    in_maps = _make_in_maps(x, w_qkv, b_qkv)
    for c in range(N_CORES):
        _, hg = divmod(c, HPC)
        j0 = DJ * hg
        in_maps[c]["wp"] = np.ascontiguousarray(w_proj[j0 : j0 + DJ, :],
                                                dtype=np.float32)

    results = _run_device(_concat_inputs(in_maps))

    out = np.zeros((B, T, E), dtype=np.float32)
    for c in range(N_CORES):
        out[c // HPC] += results[c]["out"]
    # fold b_v through the projection; b_k cancels inside softmax
    bias = b_proj + b_qkv[2 * E :] @ w_proj
    out += bias[None, None, :]
    return out


# revision 6
# speedup vs baseline: 107062.5680x; 2160.3812x over previous
"""Causal self-attention (B=2, T=2048, E=1024, 16 heads) on 8 TRN2 NeuronCores.

Sharding (Megatron-style, zero device-side collectives):
  core c in 0..7 -> batch b = c//4, head group hg = c%4 (4 heads, 256 head-dims).
  Each core computes, for its batch and its 4 heads:
    qT/kT = (w_q|w_k)^T x^T   (transposed layout: [head_dim, T])
    v     = x w_v             (natural layout: [T, head_dim], + ones column)
    sT    = kT^T-block matmuls -> [tk, tq] score blocks (causal blocks only)
    expS  = exp(sT/8) * mask  (no max-subtraction: scores are O(1) by construction)
    yT    = v_plus^T @ expS   -> [65, tq]; row 64 accumulates the softmax row-sums
    y_norm= yT[0:64] * broadcast(1/rowsum)   (broadcast via K=1 matmul on PE)
    out_c = y_norm^T w_proj[rows of its heads]  -> partial [T, E]
  Host: out[b] = sum of the 4 partials + b_proj + b_v @ w_proj.
  b_k is dropped (softmax is invariant to per-row constants); b_q is applied
  on-device via the activation bias port; b_v is folded into the output bias.

All matmuls run in float32r (TF32-like, 1 cycle/row at N>=256; ~1.5e-4 rel err).
"""

import numpy as np

N_HEAD = 16
E = 1024
B, T = 2, 2048
HD = E // N_HEAD          # 64
N_CORES = 8
HPC = 4                   # heads per core
DJ = HPC * HD             # 256 head-dim columns per core
ET = E // 128             # 8  e-tiles
TT = T // 128             # 16 t-tiles
TC = T // 512             # 4  t-chunks
SCALE = 1.0 / np.sqrt(HD)  # 0.125

_STATE = {}


def _build_nc():
    import concourse.tile as tile
    from concourse import mybir
    from concourse.bacc import Bacc

    f32 = mybir.dt.float32
    f32r = mybir.dt.float32r
    AF = mybir.ActivationFunctionType

    nc = Bacc()
    xT_d = nc.dram_tensor("xT", [E, T], f32r, kind="ExternalInput")
    wqk_d = nc.dram_tensor("wqk", [E, 2 * DJ], f32r, kind="ExternalInput")
    wv_d = nc.dram_tensor("wv", [E, DJ], f32r, kind="ExternalInput")
    wp_d = nc.dram_tensor("wp", [DJ, E], f32r, kind="ExternalInput")
    bq_d = nc.dram_tensor("bq", [128, 2], f32, kind="ExternalInput")
    mask_d = nc.dram_tensor("mask", [4, 128, 512], f32, kind="ExternalInput")
    ones4_d = nc.dram_tensor("ones4", [128, HPC, 1], f32r, kind="ExternalInput")
    ones1_d = nc.dram_tensor("ones1", [1, 128], f32r, kind="ExternalInput")
    out_d = nc.dram_tensor("out", [T, E], f32, kind="ExternalOutput")

    with tile.TileContext(nc) as tc:
        with (
            tc.tile_pool(name="xw", bufs=1) as xw,          # persistent inputs
            tc.tile_pool(name="qkv", bufs=1) as qkv,        # persistent qT/kT/v/yT
            tc.tile_pool(name="es", bufs=6) as esp,         # exp(score) blocks
            tc.tile_pool(name="nrm", bufs=2) as nrm,        # norm scratch
            tc.tile_pool(name="ob", bufs=3) as obp,         # output staging
            tc.tile_pool(name="ps", bufs=3, space="PSUM") as ps,
            tc.tile_pool(name="psy", bufs=2, space="PSUM") as psy,
        ):
            # ---- load persistent inputs ----
            xT_sb = []
            wqk_sb = []
            wv_sb = []
            for et in range(ET):
                t = xw.tile([128, T], f32r, tag=f"xT{et}")
                nc.sync.dma_start(t[:], xT_d[128 * et : 128 * (et + 1), :])
                xT_sb.append(t)
                t = xw.tile([128, 2 * DJ], f32r, tag=f"wqk{et}")
                nc.sync.dma_start(t[:], wqk_d[128 * et : 128 * (et + 1), :])
                wqk_sb.append(t)
                t = xw.tile([128, DJ], f32r, tag=f"wv{et}")
                nc.sync.dma_start(t[:], wv_d[128 * et : 128 * (et + 1), :])
                wv_sb.append(t)
            wp_sb = []
            for kt in range(2):
                t = xw.tile([128, E], f32r, tag=f"wp{kt}")
                nc.sync.dma_start(t[:], wp_d[128 * kt : 128 * (kt + 1), :])
                wp_sb.append(t)
            bq_sb = xw.tile([128, 2], f32, tag="bq")
            nc.sync.dma_start(bq_sb[:], bq_d[:])
            mask_sb = []
            for m in range(4):
                t = xw.tile([128, 512], f32, tag=f"mask{m}")
                nc.sync.dma_start(t[:], mask_d[m])
                mask_sb.append(t)
            ones4_sb = xw.tile([128, HPC, 1], f32r, tag="ones4")
            nc.sync.dma_start(ones4_sb[:], ones4_d[:])
            ones1_sb = xw.tile([1, 128], f32r, tag="ones1")
            nc.sync.dma_start(ones1_sb[:], ones1_d[:])

            # persistent intermediates
            qT_sb = [qkv.tile([128, T], f32r, tag=f"qT{i}", name=f"qT{i}") for i in range(2)]
            kT_sb = [qkv.tile([128, T], f32r, tag=f"kT{i}", name=f"kT{i}") for i in range(2)]
            v_sb = [qkv.tile([128, HPC, HD + 1], f32r, tag=f"v{i}", name=f"v{i}") for i in range(TT)]
            yT_sb = [qkv.tile([128, T], f32r, tag=f"yT{i}", name=f"yT{i}") for i in range(2)]

            # ---- phase 1: qT / kT  (transposed projections) ----
            # psum [j=128, t=512] = sum_e wqk[e, j-tile]^T @ xT[e, t-chunk]
            for jt in range(4):          # 0,1 -> q ; 2,3 -> k
                for ci in range(TC):
                    acc = ps.tile([128, 512], f32, tag="mm")
                    for et in range(ET):
                        nc.tensor.matmul(
                            acc[:],
                            wqk_sb[et][:, 128 * jt : 128 * (jt + 1)],
                            xT_sb[et][:, 512 * ci : 512 * (ci + 1)],
                            start=(et == 0),
                            stop=(et == ET - 1),
                        )
                    if jt < 2:
                        # q: add bias while copying out of PSUM (ACT engine)
                        nc.scalar.activation(
                            out=qT_sb[jt][:, 512 * ci : 512 * (ci + 1)],
                            in_=acc[:],
                            func=AF.Identity,
                            bias=bq_sb[:, jt : jt + 1],
                        )
                    else:
                        nc.vector.tensor_copy(
                            kT_sb[jt - 2][:, 512 * ci : 512 * (ci + 1)], acc[:]
                        )

            # ---- phase 2: v (natural layout) + ones column ----
            for tt in range(TT):
                acc = ps.tile([128, DJ], f32, tag="mm")
                for et in range(ET):
                    nc.tensor.matmul(
                        acc[:],
                        xT_sb[et][:, 128 * tt : 128 * (tt + 1)],
                        wv_sb[et][:],
                        start=(et == 0),
                        stop=(et == ET - 1),
                    )
                nc.vector.tensor_copy(
                    v_sb[tt][:, :, 0:HD],
                    acc[:].rearrange("p (h d) -> p h d", h=HPC),
                )
                nc.vector.tensor_copy(v_sb[tt][:, :, HD : HD + 1], ones4_sb[:])

            # ---- phase 3: attention per (head, tq-chunk) ----
            for h in range(HPC):
                kth = kT_sb[h // 2]
                qth = qT_sb[h // 2]
                r0 = HD * (h % 2)
                for ci in range(TC):
                    nj = 4 * ci + 4
                    yacc = psy.tile([HD + 1, 512], f32, tag="y")
                    for j in range(nj):
                        sacc = ps.tile([128, 512], f32, tag="s")
                        nc.tensor.matmul(
                            sacc[:],
                            kth[r0 : r0 + HD, 128 * j : 128 * (j + 1)],
                            qth[r0 : r0 + HD, 512 * ci : 512 * (ci + 1)],
                        )
                        es = esp.tile([128, 512], f32r, tag="es")
                        nc.scalar.activation(
                            out=es[:], in_=sacc[:], func=AF.Exp, scale=float(SCALE)
                        )
                        if j >= 4 * ci:
                            nc.vector.tensor_mul(es[:], es[:], mask_sb[j - 4 * ci][:])
                        nc.tensor.matmul(
                            yacc[:],
                            v_sb[j][:, h, :],
                            es[:],
                            start=(j == 0),
                            stop=(j == nj - 1),
                        )
                    # normalize: yT[0:64] * (1/rowsum) ; broadcast via K=1 matmul
                    rrow = nrm.tile([1, 512], f32r, tag="rr")
                    with nc.allow_low_precision(reason="softmax reciprocal"):
                        nc.vector.reciprocal(rrow[:], yacc[HD : HD + 1, :])
                    bacc = ps.tile([HD, 512], f32, tag="s")
                    nc.tensor.matmul(bacc[:], ones1_sb[:, 0:HD], rrow[:])
                    bs = nrm.tile([HD, 512], f32, tag="bs")
                    nc.vector.tensor_copy(bs[:], bacc[:])
                    nc.vector.tensor_mul(
                        yT_sb[h // 2][r0 : r0 + HD, 512 * ci : 512 * (ci + 1)],
                        yacc[0:HD, :],
                        bs[:],
                    )

            # ---- phase 4: projection ----
            for tt in range(TT):
                ob = obp.tile([128, E], f32, tag="ob")
                for nk in range(2):
                    acc = ps.tile([128, 512], f32, tag="mm")
                    for kt in range(2):
                        nc.tensor.matmul(
                            acc[:],
                            yT_sb[kt][:, 128 * tt : 128 * (tt + 1)],
                            wp_sb[kt][:, 512 * nk : 512 * (nk + 1)],
                            start=(kt == 0),
                            stop=(kt == 1),
                        )
                    nc.vector.tensor_copy(ob[:, 512 * nk : 512 * (nk + 1)], acc[:])
                nc.sync.dma_start(out_d[128 * tt : 128 * (tt + 1), :], ob[:])

    nc.finalize()
    return nc


def _host_constants():
    # diagonal causal masks: mask[m][r, c] = 1.0 if c >= r + 128*m else 0
    masks = np.zeros((4, 128, 512), dtype=np.float32)
    r = np.arange(128)[:, None]
    c = np.arange(512)[None, :]
    for m in range(4):
        masks[m] = (c >= r + 128 * m).astype(np.float32)
    ones4 = np.ones((128, HPC, 1), dtype=np.float32)
    ones1 = np.ones((1, 128), dtype=np.float32)
    return masks, ones4, ones1


def _make_in_maps(x, w_qkv, b_qkv):
    masks, ones4, ones1 = _host_constants()
    in_maps = []
    for c in range(N_CORES):
        b, hg = divmod(c, HPC)
        j0 = DJ * hg
        xT = np.ascontiguousarray(np.asarray(x[b], dtype=np.float32).T)
        wq = w_qkv[:, j0 : j0 + DJ]
        wk = w_qkv[:, E + j0 : E + j0 + DJ]
        wqk = np.ascontiguousarray(
            np.concatenate([wq, wk], axis=1), dtype=np.float32
        )
        wv = np.ascontiguousarray(w_qkv[:, 2 * E + j0 : 2 * E + j0 + DJ],
                                  dtype=np.float32)
        bq = np.ascontiguousarray(
            np.asarray(b_qkv[j0 : j0 + DJ], dtype=np.float32).reshape(2, 128).T
        )
        in_maps.append(
            {
                "xT": xT,
                "wqk": wqk,
                "wv": wv,
                "wp": None,  # filled below (needs w_proj)
                "bq": bq,
                "mask": masks,
                "ones4": ones4,
                "ones1": ones1,
            }
        )
    return in_maps


def _get_exec():
    """Build the Bass module and a cached jitted SPMD callable (once)."""
    if "exec" in _STATE:
        return _STATE["exec"]

    import jax
    from concourse import bass2jax, mybir
    from jax.experimental.shard_map import shard_map
    from jax.sharding import Mesh, PartitionSpec

    nc = _build_nc()
    bass2jax.install_neuronx_cc_hook()

    partition_name = (
        nc.partition_id_tensor.name if nc.partition_id_tensor else None
    )
    in_names = []
    out_names = []
    out_avals = []
    zero_outs = []
    for alloc in nc.m.functions[0].allocations:
        if not isinstance(alloc, mybir.MemoryLocationSet):
            continue
        name = alloc.memorylocations[0].name
        if alloc.kind == "ExternalInput":
            if name != partition_name:
                in_names.append(name)
        elif alloc.kind == "ExternalOutput":
            shape = tuple(alloc.tensor_shape)
            dtype = mybir.dt.np(alloc.dtype)
            out_names.append(name)
            out_avals.append(jax.core.ShapedArray(shape, dtype))
            zero_outs.append(np.zeros(shape, dtype))
    n_params = len(in_names)
    all_names = in_names + out_names
    if partition_name is not None:
        all_names = all_names + [partition_name]

    def _make_body(k):
        def _body(*args):
            operands = list(args)
            if partition_name is not None:
                operands.append(bass2jax.partition_id_tensor())
            for _ in range(k):
                outs = bass2jax._bass_exec_p.bind(
                    *operands,
                    out_avals=tuple(out_avals),
                    in_names=tuple(all_names),
                    out_names=tuple(out_names),
                    lowering_input_output_aliases=(),
                    sim_require_finite=True,
                    sim_require_nnan=True,
                    nc=nc,
                )
            return tuple(outs)

        return _body

    devices = jax.devices()[:N_CORES]
    mesh = Mesh(np.asarray(devices), ("core",))
    n_all = n_params + len(out_names)

    def _make_sharded(k):
        return jax.jit(
            shard_map(
                _make_body(k),
                mesh=mesh,
                in_specs=(PartitionSpec("core"),) * n_all,
                out_specs=(PartitionSpec("core"),) * len(out_names),
                check_rep=False,
            ),
            keep_unused=True,
        )

    sharded = _make_sharded(1)

    state = {
        "make_sharded": _make_sharded,
        "jax": jax,
        "sharded": sharded,
        "in_names": in_names,
        "out_names": out_names,
        "out_avals": out_avals,
        "zeros_dev": [
            jax.device_put(
                np.zeros((N_CORES * z.shape[0], *z.shape[1:]), z.dtype)
            )
            for z in zero_outs
        ],
    }
    _STATE["exec"] = state
    return state


def _concat_inputs(in_maps):
    st = _get_exec()
    return [
        np.concatenate([np.asarray(in_maps[c][name]) for c in range(N_CORES)], axis=0)
        for name in st["in_names"]
    ]


def _run_device(concat_in):
    """concat_in: list of global (8*dim0, ...) arrays (np or jax). Returns
    list of per-core output dicts."""
    st = _get_exec()
    out_arrs = st["sharded"](*concat_in, *st["zeros_dev"])
    res = []
    for c in range(N_CORES):
        d = {}
        for i, name in enumerate(st["out_names"]):
            shp = st["out_avals"][i].shape
            d[name] = np.asarray(out_arrs[i]).reshape(N_CORES, *shp)[c]
        res.append(d)
    return res


def kernel(x, w_qkv, b_qkv, w_proj, b_proj):
    x = np.asarray(x, dtype=np.float32)
    w_qkv = np.asarray(w_qkv, dtype=np.float32)
    b_qkv = np.asarray(b_qkv, dtype=np.float32)
    w_proj = np.asarray(w_proj, dtype=np.float32)
    b_proj = np.asarray(b_proj, dtype=np.float32)

    in_maps = _make_in_maps(x, w_qkv, b_qkv)
    for c in range(N_CORES):
        _, hg = divmod(c, HPC)
        j0 = DJ * hg
        in_maps[c]["wp"] = np.ascontiguousarray(w_proj[j0 : j0 + DJ, :],
                                                dtype=np.float32)

    results = _run_device(_concat_inputs(in_maps))

    out = np.zeros((B, T, E), dtype=np.float32)
    for c in range(N_CORES):
        out[c // HPC] += results[c]["out"]
    # fold b_v through the projection; b_k cancels inside softmax
    bias = b_proj + b_qkv[2 * E :] @ w_proj
    out += bias[None, None, :]
    return out
